# revision 12
# baseline (speedup 1.0000x reference)
"""Trainium2 Bass kernel for GaussianDDKernel.

Computes out[i,j] = (d/s^4 - 1/s^2) * exp(-d/(2 s^2)) with
d = ||x_i - y_j||^2, for x:[8192,64], y:[8192,64], sigma scalar.

Strategy (8 NeuronCores, SPMD):
  - Shard rows of x across cores (1024 rows each); replicate y.
  - Host-side: fold everything into ONE matmul contraction via augmented
    vectors:
      a = 1/sigma^2
      u_i = [-2 a^2 x_i, a^2 ||x_i||^2, 1]           (66 dims)
      v_j = [y_j,        1,             a^2 ||y_j||^2 - a]
      g[i,j] = u_i . v_j = a^2 d - a                 <- the polynomial factor
    For PE speed + fp32-level accuracy, split u,v into bf16 hi/lo parts and
    contract [u_hi; u_lo; u_hi] . [v_hi; v_hi; v_lo]  (K = 198 = 128 + 70,
    two accumulating matmuls; bf16 products are exact in fp32 PSUM).
  - Device per tile:  PE matmuls -> PSUM g
                      ACT: k = Exp(g * (-1/(2a)) + (-1/2)) = exp(-d/(2 s^2))
                      DVE: out = g * k
                      DMA out.
"""

import numpy as np

N, M, D = 8192, 8192, 64
NCORES = 8
NS = N // NCORES          # 1024 rows of x per core
KAUG = D + 2              # 66
KTOT = 3 * KAUG           # 198
KA, KB = 128, KTOT - 128  # split across two matmuls
MT = 128                  # output rows per tile (PSUM partitions)
FT = 1024                 # output cols per tile (2 PSUM banks)
MM_F = 512                # matmul moving free dim (1 PSUM bank)

_CACHE = {}


def _build(scale_exp, ft=FT, psum_bufs=4, sb_bufs=6, out_dma_split=1):
    import concourse.tile as tile
    from concourse import bacc, mybir
    from contextlib import ExitStack

    f32 = mybir.dt.float32
    bf16 = mybir.dt.bfloat16

    nc = bacc.Bacc("TRN2", target_bir_lowering=False, debug=False,
                   num_devices=NCORES)
    xa = nc.dram_tensor("xa", [KA, NS], bf16, kind="ExternalInput")
    xb = nc.dram_tensor("xb", [KB, NS], bf16, kind="ExternalInput")
    ya = nc.dram_tensor("ya", [KA, M], bf16, kind="ExternalInput")
    yb = nc.dram_tensor("yb", [KB, M], bf16, kind="ExternalInput")
    out = nc.dram_tensor("out", [NS, M], f32, kind="ExternalOutput")

    with ExitStack() as ctx:
        tc = ctx.enter_context(tile.TileContext(nc))
        const_pool = ctx.enter_context(tc.tile_pool(name="const", bufs=1))
        psum_pool = ctx.enter_context(tc.tile_pool(name="psum", bufs=psum_bufs, space="PSUM"))
        sb_pool = ctx.enter_context(tc.tile_pool(name="sb", bufs=sb_bufs))

        xa_sb = const_pool.tile([KA, NS], bf16, tag="xa")
        nc.sync.dma_start(xa_sb[:], xa.ap())
        xb_sb = const_pool.tile([KB, NS], bf16, tag="xb")
        nc.sync.dma_start(xb_sb[:], xb.ap())
        ya_sb = const_pool.tile([KA, M], bf16, tag="ya")
        nc.sync.dma_start(ya_sb[:], ya.ap())
        yb_sb = const_pool.tile([KB, M], bf16, tag="yb")
        nc.sync.dma_start(yb_sb[:], yb.ap())
        bias_sb = const_pool.tile([MT, 1], f32, tag="bias")
        nc.vector.memset(bias_sb[:], -0.5)

        for m in range(NS // MT):          # row blocks
            lhsA = xa_sb[:, m * MT:(m + 1) * MT]
            lhsB = xb_sb[:, m * MT:(m + 1) * MT]
            for f in range(M // ft):       # col blocks
                g_ps = psum_pool.tile([MT, ft], f32, tag="g")
                for s in range(ft // MM_F):
                    c0 = f * ft + s * MM_F
                    nc.tensor.matmul(
                        g_ps[:, s * MM_F:(s + 1) * MM_F],
                        lhsA, ya_sb[:, c0:c0 + MM_F],
                        start=True, stop=False)
                    nc.tensor.matmul(
                        g_ps[:, s * MM_F:(s + 1) * MM_F],
                        lhsB, yb_sb[:, c0:c0 + MM_F],
                        start=False, stop=True)
                k_sb = sb_pool.tile([MT, ft], f32, tag="k")
                nc.scalar.activation(k_sb[:], g_ps[:],
                                     mybir.ActivationFunctionType.Exp,
                                     bias=bias_sb[:], scale=float(scale_exp))
                o_sb = sb_pool.tile([MT, ft], f32, tag="o")
                nc.vector.tensor_mul(o_sb[:], k_sb[:], g_ps[:])
                w = ft // out_dma_split
                for d in range(out_dma_split):
                    nc.sync.dma_start(
                        out.ap()[m * MT:(m + 1) * MT,
                                 f * ft + d * w:f * ft + (d + 1) * w],
                        o_sb[:, d * w:(d + 1) * w])
    nc.finalize()
    return nc


def _prep_inputs(x, y, sigma):
    import ml_dtypes

    x = np.asarray(x, dtype=np.float32)
    y = np.asarray(y, dtype=np.float32)
    a = 1.0 / (float(np.asarray(sigma)) ** 2)

    x_sq = np.sum(x * x, axis=1)            # [N]
    y_sq = np.sum(y * y, axis=1)            # [M]

    ut = np.empty((KAUG, N), dtype=np.float32)
    ut[:D] = (-2.0 * a * a) * x.T
    ut[D] = (a * a) * x_sq
    ut[D + 1] = 1.0

    vt = np.empty((KAUG, M), dtype=np.float32)
    vt[:D] = y.T
    vt[D] = 1.0
    vt[D + 1] = (a * a) * y_sq - a

    bf = ml_dtypes.bfloat16
    ut_hi = ut.astype(bf)
    ut_lo = (ut - ut_hi.astype(np.float32)).astype(bf)
    vt_hi = vt.astype(bf)
    vt_lo = (vt - vt_hi.astype(np.float32)).astype(bf)

    # contraction layout: [u_hi; u_lo; u_hi] . [v_hi; v_hi; v_lo]
    xstk = np.concatenate([ut_hi, ut_lo, ut_hi], axis=0)   # [198, N]
    ystk = np.concatenate([vt_hi, vt_hi, vt_lo], axis=0)   # [198, M]

    scale_exp = -1.0 / (2.0 * a)
    return xstk, ystk, scale_exp


def _run(x, y, sigma, trace=False, tmpdir=None):
    from concourse.bass_utils import run_bass_kernel_spmd

    xstk, ystk, scale_exp = _prep_inputs(x, y, sigma)

    key = (float(scale_exp),)
    if key not in _CACHE:
        _CACHE[key] = _build(scale_exp)
    nc = _CACHE[key]

    ya_np = np.ascontiguousarray(ystk[:KA])
    yb_np = np.ascontiguousarray(ystk[KA:])
    in_maps = [
        {
            "xa": np.ascontiguousarray(xstk[:KA, c * NS:(c + 1) * NS]),
            "xb": np.ascontiguousarray(xstk[KA:, c * NS:(c + 1) * NS]),
            "ya": ya_np,
            "yb": yb_np,
        }
        for c in range(NCORES)
    ]
    res = run_bass_kernel_spmd(nc, in_maps, core_ids=list(range(NCORES)),
                               trace=trace, tmpdir=tmpdir)
    full = np.concatenate([res.results[c]["out"] for c in range(NCORES)], axis=0)
    return full, res


def kernel(x, y, sigma):
    full, _ = _run(x, y, sigma, trace=False)
    return full


# revision 13
# speedup vs baseline: 1.0536x; 1.0536x over previous
"""Trainium2 Bass kernel for GaussianDDKernel.

Computes out[i,j] = (d/s^4 - 1/s^2) * exp(-d/(2 s^2)) with
d = ||x_i - y_j||^2, for x:[8192,64], y:[8192,64], sigma scalar.

Strategy (8 NeuronCores, SPMD):
  - Shard rows of x across cores (1024 rows each); replicate y.
  - Host-side: fold everything into ONE matmul contraction via augmented
    vectors:
      a = 1/sigma^2
      u_i = [-2 a^2 x_i, a^2 ||x_i||^2, 1]           (66 dims)
      v_j = [y_j,        1,             a^2 ||y_j||^2 - a]
      g[i,j] = u_i . v_j = a^2 d - a                 <- the polynomial factor
    For PE speed + fp32-level accuracy, split u,v into bf16 hi/lo parts and
    contract [u_hi; u_lo; u_hi] . [v_hi; v_hi; v_lo]  (K = 198 = 128 + 70,
    two accumulating matmuls; bf16 products are exact in fp32 PSUM).
  - Device per tile:  PE matmuls -> PSUM g
                      ACT: k = Exp(g * (-1/(2a)) + (-1/2)) = exp(-d/(2 s^2))
                      DVE: out = g * k
                      DMA out.
"""

import numpy as np

N, M, D = 8192, 8192, 64
NCORES = 8
NS = N // NCORES          # 1024 rows of x per core
KAUG = D + 2              # 66
KTOT = 3 * KAUG           # 198
KA, KB = 128, KTOT - 128  # split across two matmuls
MT = 128                  # output rows per tile (PSUM partitions)
FT = 1024                 # output cols per tile (2 PSUM banks)
MM_F = 512                # matmul moving free dim (1 PSUM bank)

_CACHE = {}


def _build(scale_exp, ft=FT, psum_bufs=4, sb_bufs=6, out_dma_split=1):
    import concourse.tile as tile
    from concourse import bacc, mybir
    from contextlib import ExitStack

    f32 = mybir.dt.float32
    bf16 = mybir.dt.bfloat16

    nc = bacc.Bacc("TRN2", target_bir_lowering=False, debug=False,
                   num_devices=NCORES)
    xa = nc.dram_tensor("xa", [KA, NS], bf16, kind="ExternalInput")
    xb = nc.dram_tensor("xb", [KB, NS], bf16, kind="ExternalInput")
    ya = nc.dram_tensor("ya", [KA, M], bf16, kind="ExternalInput")
    yb = nc.dram_tensor("yb", [KB, M], bf16, kind="ExternalInput")
    out = nc.dram_tensor("out", [NS, M], f32, kind="ExternalOutput")

    with ExitStack() as ctx:
        tc = ctx.enter_context(tile.TileContext(nc))
        const_pool = ctx.enter_context(tc.tile_pool(name="const", bufs=1))
        psum_pool = ctx.enter_context(tc.tile_pool(name="psum", bufs=psum_bufs, space="PSUM"))
        sb_pool = ctx.enter_context(tc.tile_pool(name="sb", bufs=sb_bufs))

        xa_sb = const_pool.tile([KA, NS], bf16, tag="xa")
        nc.sync.dma_start(xa_sb[:], xa.ap())
        xb_sb = const_pool.tile([KB, NS], bf16, tag="xb")
        nc.sync.dma_start(xb_sb[:], xb.ap())
        # Load y in column chunks so the first matmuls start as soon as the
        # first slice lands instead of waiting for the full 3.2 MB.
        ya_sb = const_pool.tile([KA, M], bf16, tag="ya")
        yb_sb = const_pool.tile([KB, M], bf16, tag="yb")
        for c in range(M // ft):
            sl = slice(c * ft, (c + 1) * ft)
            nc.sync.dma_start(ya_sb[:, sl], ya.ap()[:, sl])
            nc.sync.dma_start(yb_sb[:, sl], yb.ap()[:, sl])
        bias_sb = const_pool.tile([MT, 1], f32, tag="bias")
        nc.vector.memset(bias_sb[:], -0.5)

        for m in range(NS // MT):          # row blocks
            lhsA = xa_sb[:, m * MT:(m + 1) * MT]
            lhsB = xb_sb[:, m * MT:(m + 1) * MT]
            for f in range(M // ft):       # col blocks
                g_ps = psum_pool.tile([MT, ft], f32, tag="g")
                for s in range(ft // MM_F):
                    c0 = f * ft + s * MM_F
                    nc.tensor.matmul(
                        g_ps[:, s * MM_F:(s + 1) * MM_F],
                        lhsA, ya_sb[:, c0:c0 + MM_F],
                        start=True, stop=False)
                    nc.tensor.matmul(
                        g_ps[:, s * MM_F:(s + 1) * MM_F],
                        lhsB, yb_sb[:, c0:c0 + MM_F],
                        start=False, stop=True)
                k_sb = sb_pool.tile([MT, ft], f32, tag="k")
                nc.scalar.activation(k_sb[:], g_ps[:],
                                     mybir.ActivationFunctionType.Exp,
                                     bias=bias_sb[:], scale=float(scale_exp))
                o_sb = sb_pool.tile([MT, ft], f32, tag="o")
                nc.vector.tensor_mul(o_sb[:], k_sb[:], g_ps[:])
                w = ft // out_dma_split
                for d in range(out_dma_split):
                    nc.sync.dma_start(
                        out.ap()[m * MT:(m + 1) * MT,
                                 f * ft + d * w:f * ft + (d + 1) * w],
                        o_sb[:, d * w:(d + 1) * w])
    nc.finalize()
    return nc


def _prep_inputs(x, y, sigma):
    import ml_dtypes

    x = np.asarray(x, dtype=np.float32)
    y = np.asarray(y, dtype=np.float32)
    a = 1.0 / (float(np.asarray(sigma)) ** 2)

    x_sq = np.sum(x * x, axis=1)            # [N]
    y_sq = np.sum(y * y, axis=1)            # [M]

    ut = np.empty((KAUG, N), dtype=np.float32)
    ut[:D] = (-2.0 * a * a) * x.T
    ut[D] = (a * a) * x_sq
    ut[D + 1] = 1.0

    vt = np.empty((KAUG, M), dtype=np.float32)
    vt[:D] = y.T
    vt[D] = 1.0
    vt[D + 1] = (a * a) * y_sq - a

    bf = ml_dtypes.bfloat16
    ut_hi = ut.astype(bf)
    ut_lo = (ut - ut_hi.astype(np.float32)).astype(bf)
    vt_hi = vt.astype(bf)
    vt_lo = (vt - vt_hi.astype(np.float32)).astype(bf)

    # contraction layout: [u_hi; u_lo; u_hi] . [v_hi; v_hi; v_lo]
    xstk = np.concatenate([ut_hi, ut_lo, ut_hi], axis=0)   # [198, N]
    ystk = np.concatenate([vt_hi, vt_hi, vt_lo], axis=0)   # [198, M]

    scale_exp = -1.0 / (2.0 * a)
    return xstk, ystk, scale_exp


def _run(x, y, sigma, trace=False, tmpdir=None):
    from concourse.bass_utils import run_bass_kernel_spmd

    xstk, ystk, scale_exp = _prep_inputs(x, y, sigma)

    key = (float(scale_exp),)
    if key not in _CACHE:
        _CACHE[key] = _build(scale_exp)
    nc = _CACHE[key]

    ya_np = np.ascontiguousarray(ystk[:KA])
    yb_np = np.ascontiguousarray(ystk[KA:])
    in_maps = [
        {
            "xa": np.ascontiguousarray(xstk[:KA, c * NS:(c + 1) * NS]),
            "xb": np.ascontiguousarray(xstk[KA:, c * NS:(c + 1) * NS]),
            "ya": ya_np,
            "yb": yb_np,
        }
        for c in range(NCORES)
    ]
    res = run_bass_kernel_spmd(nc, in_maps, core_ids=list(range(NCORES)),
                               trace=trace, tmpdir=tmpdir)
    full = np.concatenate([res.results[c]["out"] for c in range(NCORES)], axis=0)
    return full, res


def kernel(x, y, sigma):
    full, _ = _run(x, y, sigma, trace=False)
    return full
